# revision 1
# baseline (speedup 1.0000x reference)
"""Trainium2 Bass kernel for nn_BertSelfAttention_64476049047801.

Math notes (vs the jax reference):
  - The rel-shift of the Gaussian branch reduces analytically to
      g[i, j] = exp(-(j-i)^2 * a_i),  a_i = 1/(2*exp(2*log_s_i) + 1e-12)
    (the -log_s term is constant per row and cancels in the row
    normalization).  The exponent -(j-i)^2 a_i = 1*(-i^2 a_i) + j*(2 i a_i)
    + j^2*(-a_i) is rank-3 in (j-basis x i-coefficients), so it is
    generated on the tensor engine with a K=3 matmul against the constant
    basis [1, j, j^2].
  - Softmax needs no max subtraction (scores ~ N(0,1)).
  - Row sums of exp-scores come for free as a 65th output row of the
    PV matmul (lhsT = [v | ones]).
  - The 2-way mixing softmax is computed as sigmoid(w_t - w_s) via
    exp/reciprocal (stays in the "exp" ACT table set).

Sharding: data-parallel over batch B=8 across the 8 cores; each core
computes one full batch element.  No collectives; host scatters/gathers.
All compute layouts are transposed (hid, seq); the host transposes the
final (512, 1024) per-core output back to (1024, 512).
"""

import sys, os

sys.path.insert(0, "/opt/trn_rl_repo")
PHASES = int(os.environ.get("PHASES", "99"))

import numpy as np

import concourse.bass as bass
import concourse.bacc as bacc
import concourse.mybir as mybir
from concourse import tile
from concourse.tile import TileContext

B, S, HID, NH, D = 8, 1024, 512, 8, 64
P = 128
KC = HID // P  # 4 hid chunks
SC = S // P  # 8 seq chunks
FP = mybir.dt.float32
FR = mybir.dt.float32r
AF = mybir.ActivationFunctionType
OP = mybir.AluOpType

N_CORES = 8


def _mm(nc, out, lhsT, rhs, start, stop):
    nc.tensor.matmul(out, lhsT, rhs, start=start, stop=stop)


def build_nc():
    nc = bacc.Bacc(None)
    _build_body(nc)
    nc.finalize()
    return nc


def _build_body(nc):

    # ---- DRAM I/O ----
    xT_d = nc.dram_tensor("xT", [HID, S], FR, kind="ExternalInput")
    W_d = {
        n: nc.dram_tensor(n, [HID, HID], FR, kind="ExternalInput")
        for n in ("Wq", "Wk", "Wtv", "Wsv", "Wmap")
    }
    Wsig_d = nc.dram_tensor("Wsig", [HID, NH], FR, kind="ExternalInput")
    bq_d = nc.dram_tensor("bq", [HID, 1], FP, kind="ExternalInput")
    bk_d = nc.dram_tensor("bk", [HID, 1], FP, kind="ExternalInput")
    bmap_d = nc.dram_tensor("bmap", [HID, 1], FP, kind="ExternalInput")
    bsig2_d = nc.dram_tensor("bsig2", [NH, 1], FP, kind="ExternalInput")
    btvr_d = nc.dram_tensor("btv_r", [1, HID], FR, kind="ExternalInput")
    bsvr_d = nc.dram_tensor("bsv_r", [1, HID], FR, kind="ExternalInput")
    aq_d = nc.dram_tensor("attn_q", [HID, 1], FR, kind="ExternalInput")
    D2_d = nc.dram_tensor("D2band", [P, 1920], FP, kind="ExternalInput")
    HS8_d = nc.dram_tensor("HS8", [NH, HID], FR, kind="ExternalInput")
    ones_d = nc.dram_tensor("ones2d", [P, S], FR, kind="ExternalInput")
    out_d = nc.dram_tensor("outT", [HID, S], FP, kind="ExternalOutput")

    with nc.allow_low_precision(reason="fp32r matmul operand staging"), TileContext(nc) as tc:
        with (
            tc.tile_pool(name="singles", bufs=1) as singles,
            tc.tile_pool(name="b4k", bufs=6) as b4k,  # xT -> a/b/g + E -> th
            tc.tile_pool(name="qk", bufs=8) as qkp,  # qT, kT
            tc.tile_pool(name="wts", bufs=8) as wts,  # Wq..Wsv -> tT, sT
            tc.tile_pool(name="sigw", bufs=2) as sigw,
            tc.tile_pool(name="gst", bufs=2) as gstp,
            tc.tile_pool(name="ww", bufs=4) as wwp,
            tc.tile_pool(name="mixw", bufs=2) as mixw,
            tc.tile_pool(name="big", bufs=2, space="PSUM") as bigp,
            tc.tile_pool(name="pv", bufs=2, space="PSUM") as pvp,
        ):
            # ---- constant / input loads ----
            xT = []
            for c in range(KC):
                t = b4k.tile([P, S], FR, tag="b4k")
                nc.sync.dma_start(t[:], xT_d[c * P : (c + 1) * P, :])
                xT.append(t)

            def load_w(name):
                ts = []
                for c in range(KC):
                    t = wts.tile([P, HID], FR, tag="wts")
                    nc.sync.dma_start(t[:], W_d[name][c * P : (c + 1) * P, :])
                    ts.append(t)
                return ts

            Wmap_t = []
            for c in range(KC):
                t = singles.tile([P, HID], FR, tag=f"wmap{c}")
                nc.sync.dma_start(t[:], W_d["Wmap"][c * P : (c + 1) * P, :])
                Wmap_t.append(t)
            Wsig_t = []
            for c in range(KC):
                t = singles.tile([P, NH], FR, tag=f"wsig{c}")
                nc.sync.dma_start(t[:], Wsig_d[c * P : (c + 1) * P, :])
                Wsig_t.append(t)

            def load_small(dram, shape, tag, dt_=FP):
                t = singles.tile(shape, dt_, tag=tag)
                nc.sync.dma_start(t[:], dram[:])
                return t

            # biases per chunk
            bias_t = {}
            for nm, dram in (("bq", bq_d), ("bk", bk_d), ("bmap", bmap_d)):
                ts = []
                for c in range(KC):
                    t = singles.tile([P, 1], FP, tag=f"{nm}{c}")
                    nc.sync.dma_start(t[:], dram[c * P : (c + 1) * P, :])
                    ts.append(t)
                bias_t[nm] = ts
            aq_t = []
            for c in range(KC):
                t = singles.tile([P, 1], FR, tag=f"aq{c}")
                nc.sync.dma_start(t[:], aq_d[c * P : (c + 1) * P, :])
                aq_t.append(t)
            bsig2_t = load_small(bsig2_d, [NH, 1], "bsig2")
            btvr_t = load_small(btvr_d, [1, HID], "btvr", FR)
            bsvr_t = load_small(bsvr_d, [1, HID], "bsvr", FR)
            D2_t = load_small(D2_d, [P, 1920], "D2band")
            HS8_t = load_small(HS8_d, [NH, HID], "HS8", FR)
            ones_t = singles.tile([1, S], FR, tag="ones")
            nc.sync.dma_start(ones_t[:], ones_d[0:1, :])

            # ---- phase 1: projections ----
            def proj_T(Wt, bias, outs):
                # out[m] (128 hid_out, S) = W[:, m].T @ xT ; += bias per row
                for m in range(KC):
                    ps = bigp.tile([P, S], FP, tag="big")
                    for n in range(2):
                        nsl = slice(n * 512, (n + 1) * 512)
                        for k in range(KC):
                            _mm(
                                nc,
                                ps[:, nsl],
                                Wt[k][:, m * P : (m + 1) * P],
                                xT[k][:, nsl],
                                start=(k == 0),
                                stop=(k == KC - 1),
                            )
                    o = qkp.tile([P, S], FR, tag="qk")
                    nc.vector.tensor_scalar_add(o[:], ps[:], bias[m][:, 0:1])
                    outs.append(o)

            qT, kT = [], []
            if PHASES < 1:
                return
            Wq_t = load_w("Wq")
            proj_T(Wq_t, bias_t["bq"], qT)
            Wk_t = load_w("Wk")
            proj_T(Wk_t, bias_t["bk"], kT)

            # tv/sv natural (s, hid) with interleaved ones cols: [v_h | 1]*8
            def proj_nat(Wt, brow, outs, pfx):
                for m in range(SC):
                    ps = bigp.tile([P, HID], FP, tag="big")
                    for k in range(KC):
                        _mm(
                            nc,
                            ps[:],
                            xT[k][:, m * P : (m + 1) * P],
                            Wt[k][:],
                            start=(k == 0),
                            stop=False,
                        )
                    _mm(
                        nc,
                        ps[:],
                        ones_t[0:1, m * P : (m + 1) * P],
                        brow[0:1, :],
                        start=False,
                        stop=True,
                    )
                    o = singles.tile([P, NH * 65], FR, tag=f"vp{pfx}_{m}")
                    ov = o.rearrange("p (h c) -> p h c", h=NH)
                    pv_ = ps.rearrange("p (h c) -> p h c", h=NH)
                    nc.vector.tensor_copy(ov[:, :, 0:64], pv_[:])
                    nc.sync.dma_start(
                        ov[:, :, 64:65], ones_d[:, 0:NH].rearrange("p (h o) -> p h o", o=1)
                    )
                    outs.append(o)

            tvp, svp = [], []
            if PHASES < 2:
                return
            Wtv_t = load_w("Wtv")
            proj_nat(Wtv_t, btvr_t, tvp, "t")
            Wsv_t = load_w("Wsv")
            proj_nat(Wsv_t, bsvr_t, svp, "s")

            # sigma head: a_i = 1/(2*exp(2*log_s)+1e-12); C rows per head
            if PHASES < 3:
                return
            ps = bigp.tile([NH, S], FP, tag="big")
            for n in range(2):
                nsl = slice(n * 512, (n + 1) * 512)
                for k in range(KC):
                    _mm(
                        nc,
                        ps[0:NH, nsl],
                        Wsig_t[k][:],
                        xT[k][:, nsl],
                        start=(k == 0),
                        stop=(k == KC - 1),
                    )
            e2 = sigw.tile([NH, S], FP, tag="sigw")
            nc.scalar.activation(
                e2[:], ps[0:NH, :], AF.Exp, bias=bsig2_t[:, 0:1], scale=2.0
            )
            den = sigw.tile([NH, S], FP, tag="sigw")
            nc.vector.tensor_scalar(den[:], e2[:], 2.0, 1e-12, OP.mult, OP.add)
            rcp = sigw.tile([NH, S], FP, tag="sigw")
            nc.vector.reciprocal(rcp[:], den[:])
            g_t = singles.tile([NH, S], FP, tag="negA")  # -a per (head, i)
            nc.vector.tensor_scalar_mul(g_t[:], rcp[:], -1.0)

            # ---- rowsum + branch output holders ----
            rs_t = singles.tile([NH, S], FP, tag="rs_t")
            rs_s = singles.tile([NH, S], FP, tag="rs_s")
            tT = [
                wts.tile([P, S], FR, tag="wts", name=f"tT{i}") for i in range(KC)
            ]
            sT = [
                wts.tile([P, S], FR, tag="wts", name=f"sT{i}") for i in range(KC)
            ]

            def branch(vp, outT_tiles, rs_tile, gauss):
                for h in range(NH):
                    c, half = h // 2, (h % 2) * 64
                    if gauss:
                        ga = gstp.tile([1, S], FR, tag="gst")
                        nc.sync.dma_start(ga[0:1, :], g_t[h : h + 1, :].bitcast(FR))
                        abc = bigp.tile([P, S], FP, tag="big")
                        for n in range(2):
                            nsl = slice(n * 512, (n + 1) * 512)
                            _mm(
                                nc,
                                abc[:, nsl],
                                ones_t[0:1, 0:P],
                                ga[0:1, nsl],
                                start=True,
                                stop=True,
                            )
                    pv_ps = pvp.tile([65, S], FP, tag="pv")
                    for jc in range(SC):
                        E = b4k.tile([P, S], FR, tag="b4k")
                        if gauss:
                            ex = b4k.tile([P, S], FP, tag="b4k")
                            nc.vector.tensor_tensor(
                                ex[:],
                                D2_t[:, 896 - P * jc : 896 - P * jc + S],
                                abc[:],
                                OP.mult,
                            )
                            nc.scalar.activation(E[:], ex[:], AF.Exp)
                        else:
                            sp = bigp.tile([P, S], FP, tag="big")
                            for n in range(2):
                                nsl = slice(n * 512, (n + 1) * 512)
                                _mm(
                                    nc,
                                    sp[:, nsl],
                                    kT[c][half : half + 64, jc * P : (jc + 1) * P],
                                    qT[c][half : half + 64, nsl],
                                    start=True,
                                    stop=True,
                                )
                            nc.scalar.activation(E[:], sp[:], AF.Exp, scale=0.125)
                        for n in range(2):
                            nsl = slice(n * 512, (n + 1) * 512)
                            _mm(
                                nc,
                                pv_ps[:, nsl],
                                vp[jc][:, h * 65 : h * 65 + 65],
                                E[:, nsl],
                                start=(jc == 0),
                                stop=(jc == SC - 1),
                            )
                    # evacuate: d rows 0..63, rowsum row 64
                    rst = gstp.tile([1, S], FP, tag="rst")
                    nc.vector.tensor_copy(rst[0:1, :], pv_ps[64:65, :])
                    nc.sync.dma_start(rs_tile[h : h + 1, :], rst[0:1, :])
                    if h % 2 == 0:
                        nc.vector.tensor_copy(
                            outT_tiles[c][0:64, :], pv_ps[0:64, :]
                        )
                    else:
                        nc.vector.tensor_copy(
                            outT_tiles[c][64:128, :], pv_ps[0:64, :]
                        )

            if PHASES < 4:
                return
            tc.no_sync_barrier()
            branch(tvp, tT, rs_t, gauss=False)
            if PHASES < 5:
                return
            tc.no_sync_barrier()
            branch(svp, sT, rs_s, gauss=True)

            # ---- normalize by row sums ----
            if PHASES < 6:
                return
            tc.no_sync_barrier()
            for rs, XT in ((rs_t, tT), (rs_s, sT)):
                rc = sigw.tile([NH, S], FR, tag="sigw")
                nc.vector.reciprocal(rc[:], rs[:])
                for m in range(KC):
                    pb = bigp.tile([P, S], FP, tag="big")
                    for n in range(2):
                        nsl = slice(n * 512, (n + 1) * 512)
                        _mm(
                            nc,
                            pb[:, nsl],
                            HS8_t[0:NH, m * P : (m + 1) * P],
                            rc[0:NH, nsl],
                            start=True,
                            stop=True,
                        )
                    nc.vector.tensor_tensor(XT[m][:], XT[m][:], pb[:], OP.mult)

            # ---- gate: w = sigmoid(wt - ws) ----
            if PHASES < 7:
                return
            tc.no_sync_barrier()
            wl = {}
            for key, XT in (("t", tT), ("s", sT)):
                th = []
                for m in range(KC):
                    pm = bigp.tile([P, S], FP, tag="big")
                    for n in range(2):
                        nsl = slice(n * 512, (n + 1) * 512)
                        for k in range(KC):
                            _mm(
                                nc,
                                pm[:, nsl],
                                Wmap_t[k][:, m * P : (m + 1) * P],
                                XT[k][:, nsl],
                                start=(k == 0),
                                stop=(k == KC - 1),
                            )
                    t = b4k.tile([P, S], FR, tag="b4k")
                    nc.scalar.activation(
                        t[:], pm[:], AF.Tanh, bias=bias_t["bmap"][m][:, 0:1]
                    )
                    th.append(t)
                pw = bigp.tile([1, S], FP, tag="big")
                for n in range(2):
                    nsl = slice(n * 512, (n + 1) * 512)
                    for k in range(KC):
                        _mm(
                            nc,
                            pw[0:1, nsl],
                            aq_t[k][:, 0:1],
                            th[k][:, nsl],
                            start=(k == 0),
                            stop=(k == KC - 1),
                        )
                w = wwp.tile([1, S], FP, tag="ww")
                nc.vector.tensor_copy(w[:], pw[0:1, :])
                wl[key] = w

            if PHASES < 8:
                return
            wd = wwp.tile([1, S], FP, tag="ww")
            nc.vector.tensor_sub(wd[:], wl["t"][:], wl["s"][:])
            ew = wwp.tile([1, S], FP, tag="ww")
            nc.scalar.activation(ew[:], wd[:], AF.Exp, scale=-1.0)
            den = wwp.tile([1, S], FP, tag="ww")
            nc.vector.tensor_scalar_add(den[:], ew[:], 1.0)
            wgf = wwp.tile([1, S], FP, tag="ww")
            nc.vector.reciprocal(wgf[:], wgf[:] if False else den[:])
            wg = wwp.tile([1, S], FR, tag="ww")
            nc.vector.tensor_copy(wg[:], wgf[:])

            if PHASES < 9:
                return
            pwb = bigp.tile([P, S], FP, tag="big")
            for n in range(2):
                nsl = slice(n * 512, (n + 1) * 512)
                _mm(
                    nc,
                    pwb[:, nsl],
                    ones_t[0:1, 0:P],
                    wg[0:1, nsl],
                    start=True,
                    stop=True,
                )

            # ---- mix = s + w*(t - s) ----
            for m in range(KC):
                dt_ = mixw.tile([P, S], FP, tag="mixw")
                nc.vector.tensor_sub(dt_[:], tT[m][:], sT[m][:])
                nc.vector.tensor_tensor(dt_[:], dt_[:], pwb[:], OP.mult)
                nc.vector.tensor_add(dt_[:], dt_[:], sT[m][:])
                nc.sync.dma_start(out_d[m * P : (m + 1) * P, :], dt_[:])


def _prep_maps(inputs):
    f = lambda a: np.ascontiguousarray(np.asarray(a, dtype=np.float32))
    hs = f(inputs["hidden_states"])  # (B, S, HID)
    u = np.arange(P, dtype=np.float32)[:, None]
    cc = np.arange(1920, dtype=np.float32)[None, :]
    D2band = (u - cc + 896.0) ** 2  # D2band[u, c] = (128*jc + u - i)^2 at c = i - 128*jc + 896
    HS8 = np.zeros((NH, HID), np.float32)
    for h in range(NH):
        HS8[h, h * 64 : (h + 1) * 64] = 1.0
    shared = {
        "Wq": f(inputs["Wq"]),
        "Wk": f(inputs["Wk"]),
        "Wtv": f(inputs["Wtv"]),
        "Wsv": f(inputs["Wsv"]),
        "Wmap": f(inputs["Wmap"]),
        "Wsig": f(inputs["Wsig"]),
        "bq": f(inputs["bq"]).reshape(HID, 1),
        "bk": f(inputs["bk"]).reshape(HID, 1),
        "bmap": f(inputs["bmap"]).reshape(HID, 1),
        "bsig2": 2.0 * f(inputs["bsig"]).reshape(NH, 1),
        "btv_r": f(inputs["btv"]).reshape(1, HID),
        "bsv_r": f(inputs["bsv"]).reshape(1, HID),
        "attn_q": f(inputs["attn_q"]).reshape(HID, 1),
        "D2band": np.ascontiguousarray(D2band.astype(np.float32)),
        "HS8": HS8,
        "ones2d": np.ones((P, S), np.float32),
    }
    maps = []
    for b in range(N_CORES):
        m = dict(shared)
        m["xT"] = np.ascontiguousarray(hs[b].T)  # (HID, S)
        maps.append(m)
    return maps


def kernel(**inputs):
    from concourse import bass_utils

    nc = build_nc()
    maps = _prep_maps(inputs)
    res = bass_utils.run_bass_kernel_spmd(nc, maps, list(range(N_CORES)))
    out = np.stack(
        [np.asarray(res.results[b]["outT"]).T for b in range(N_CORES)]
    )
    return out.astype(np.float32)


if __name__ == "__main__":
    nc = build_nc()
    print("built ok:", nc)

